# revision 2
# baseline (speedup 1.0000x reference)
"""Trainium2 Bass kernel for nn_ContrastiveLoss (B=4096, D=512, F=128), 8 NeuronCores.

Row-sharded: core c owns rows [c*512, (c+1)*512). All cores receive the FULL
E^T / normalized-f^T (identical buffers) plus their own 512-column weight
slices, so one static NEFF serves all cores (no per-core rolls).

Math (T=0.1 -> S' = 10*S_raw):
  bce_ij = pos ? softplus(-S') : softplus(S') = softplus(S' * sgnneg)
  softplus(x) = relu(x) + ln(1+exp(-|x|));  Sum_j relu(S'*sgnneg)
    = Sum_j relu(S') - Sum_{pos j} S'                 [relu(x)-x = relu(-x)]
  The diagonal (always pos, S'_ii ~ +5000) cancels exactly in R - P, and its
  ln-term is 0 in fp32. The dropped ln(1+exp(-|S'|)) tail is bounded by
  ln2 * #(|S_raw|<~1) per row ~ 4e1 of a ~3.7e5 row sum (~1e-4 relative).

Per core, per [128 x 1024] tile (16 tiles):
  psG = sfl^T @ sfn      (PE, 2 matmuls)   [tsim block]
  psS = etl^T @ et       (PE, 8 matmuls, PSUM-accumulated over D/128)
  ACT: relu_t = Relu(psS),            accum -> R
  mask (alternating to balance engines):
    even tiles: ACT mask_t = Sign(psG - 0.5), accum -> C  (sum of +-1)
    odd  tiles: DVE mask_t = (psG is_gt 0.5), accum -> C  (count)
  DVE: p_t = mask*psS (max(sgn,0) resp. b*1), accum -> P
Host: row_bce = 10*(R - P); pos counts from C; validity + final scalar.

All accum-pass outputs are fp32: the in-instruction accumulator follows the
OUT dtype (bf16 outs lose integer counts past 256).

This walrus build caps sync waits at 1 per instruction; _split_multiwaits
legalizes the Tile-emitted BIR by hoisting extra waits onto single-wait Drains.
"""

import json
import ml_dtypes
import numpy as np
from contextlib import ExitStack

import concourse.bass as bass
import concourse.tile as tile
import concourse.mybir as mybir
from concourse.bass_utils import run_bass_kernel_spmd

f32 = mybir.dt.float32
bf16 = mybir.dt.bfloat16
AFT = mybir.ActivationFunctionType
ALU = mybir.AluOpType

B, D, F = 4096, 512, 128
NCORES = 8
RPC = B // NCORES          # 512 rows per core
NR = RPC // 128            # 4 row blocks of 128
CHUNK = 1024               # column chunk (2 PSUM banks)
NN = B // CHUNK            # 4 column chunks
NT = NR * NN               # 16 stat columns
KC = D // 128              # 4 contraction chunks
INV_T = 10.0               # 1/TEMPERATURE


def _use_sgn(idx: int) -> bool:
    """Which engine computes the tsim>0.5 mask for stat column idx."""
    return idx % 2 == 0


def _split_multiwaits(m: dict) -> int:
    """Split >1-wait instructions into single-wait Drain chains (walrus cap)."""
    n_new = 0
    for fn in m["functions"]:
        for blk in fn["blocks"]:
            out = []
            for inst in blk["instructions"]:
                si = inst.get("sync_info") or {}
                ow = si.get("on_wait") or []
                if len(ow) > 1:
                    for w in ow[:-1]:
                        n_new += 1
                        out.append({
                            "debug": inst.get("debug", 0),
                            "engine": inst["engine"],
                            "ins": [], "outs": [],
                            "is_reset_sema": False,
                            "name": f"{inst['name']}-sw{n_new}",
                            "opcode": "Drain",
                            "sync_info": {"on_update": [], "on_wait": [w]},
                        })
                    si["on_wait"] = [ow[-1]]
                out.append(inst)
            blk["instructions"] = out
    return n_new


def _build_nc() -> bass.Bass:
    nc = bass.Bass("TRN2", target_bir_lowering=False, debug=False)
    et_d = nc.dram_tensor("et", [D, B], bf16, kind="ExternalInput").ap()
    sfn_d = nc.dram_tensor("sfn", [F, B], bf16, kind="ExternalInput").ap()
    etl_d = nc.dram_tensor("etl", [D, RPC], bf16, kind="ExternalInput").ap()
    sfl_d = nc.dram_tensor("sfl", [F, RPC], bf16, kind="ExternalInput").ap()
    out_r = nc.dram_tensor("out_r", [128, NT], f32, kind="ExternalOutput").ap()
    out_p = nc.dram_tensor("out_p", [128, NT], f32, kind="ExternalOutput").ap()
    out_c = nc.dram_tensor("out_c", [128, NT], f32, kind="ExternalOutput").ap()

    with tile.TileContext(nc) as tc, ExitStack() as ctx:
        main = ctx.enter_context(tc.tile_pool(name="main", bufs=1))
        scratch = ctx.enter_context(tc.tile_pool(name="scratch", bufs=3))

        # small weight slices first so tile 0's matmuls can start ASAP
        etl_sb = [main.tile([128, RPC], bf16, name=f"etl{kc}") for kc in range(KC)]
        for kc in range(KC):
            nc.sync.dma_start(out=etl_sb[kc],
                              in_=etl_d[kc * 128:(kc + 1) * 128, :])
        sfl_sb = main.tile([F, RPC], bf16, name="sfl_sb")
        nc.sync.dma_start(out=sfl_sb, in_=sfl_d)

        # big tensors streamed column-chunk-major: chunk n4 lands before the
        # n4-th tile group needs it
        et_sb = [main.tile([128, B], bf16, name=f"et{kc}") for kc in range(KC)]
        sfn_sb = main.tile([F, B], bf16, name="sfn_sb")
        for n4 in range(NN):
            c0 = n4 * CHUNK
            nc.sync.dma_start(out=sfn_sb[:, c0:c0 + CHUNK],
                              in_=sfn_d[:, c0:c0 + CHUNK])
            for kc in range(KC):
                nc.sync.dma_start(
                    out=et_sb[kc][:, c0:c0 + CHUNK],
                    in_=et_d[kc * 128:(kc + 1) * 128, c0:c0 + CHUNK])

        neg_half = main.tile([128, 1], f32, name="neg_half")
        nc.vector.memset(neg_half, -0.5)

        r_st = main.tile([128, NT], f32, name="r_st")
        p_st = main.tile([128, NT], f32, name="p_st")
        c_st = main.tile([128, NT], f32, name="c_st")

        with tc.tile_pool(name="pp_s", bufs=2, space="PSUM") as pp_s, \
             tc.tile_pool(name="pp_g", bufs=2, space="PSUM") as pp_g:
            for n4 in range(NN):
                for r in range(NR):
                    idx = n4 * NR + r
                    c0 = n4 * CHUNK
                    # G first: its cheap mask pass overlaps the S matmuls
                    psG = pp_g.tile([128, CHUNK], f32, name="psG")
                    for h in range(2):
                        nc.tensor.matmul(
                            psG[:, h * 512:(h + 1) * 512],
                            sfl_sb[:, r * 128:(r + 1) * 128],
                            sfn_sb[:, c0 + h * 512:c0 + (h + 1) * 512],
                            start=True, stop=True)
                    psS = pp_s.tile([128, CHUNK], f32, name="psS")
                    for kc in range(KC):
                        for h in range(2):
                            nc.tensor.matmul(
                                psS[:, h * 512:(h + 1) * 512],
                                etl_sb[kc][:, r * 128:(r + 1) * 128],
                                et_sb[kc][:, c0 + h * 512:c0 + (h + 1) * 512],
                                start=(kc == 0), stop=(kc == KC - 1))

                    mask_t = scratch.tile([128, CHUNK], f32, name="mask_t")
                    if _use_sgn(idx):
                        nc.scalar.activation(mask_t, psG, AFT.Sign,
                                             bias=neg_half,
                                             accum_out=c_st[:, idx:idx + 1])
                        p_op0, p_scalar = ALU.max, 0.0
                    else:
                        nc.vector.tensor_scalar(
                            out=mask_t, in0=psG, scalar1=0.5, scalar2=0.0,
                            op0=ALU.is_gt, op1=ALU.add,
                            accum_out=c_st[:, idx:idx + 1])
                        p_op0, p_scalar = ALU.mult, 1.0

                    relu_t = scratch.tile([128, CHUNK], f32, name="relu_t")
                    nc.scalar.activation(relu_t, psS, AFT.Relu,
                                         accum_out=r_st[:, idx:idx + 1])
                    p_t = scratch.tile([128, CHUNK], f32, name="p_t")
                    nc.vector.scalar_tensor_tensor(
                        out=p_t, in0=mask_t, scalar=p_scalar, in1=psS,
                        op0=p_op0, op1=ALU.mult,
                        accum_out=p_st[:, idx:idx + 1])

        nc.sync.dma_start(out=out_r, in_=r_st)
        nc.sync.dma_start(out=out_p, in_=p_st)
        nc.sync.dma_start(out=out_c, in_=c_st)

    orig = nc.to_json_bytes

    def patched():
        m = json.loads(orig())
        _split_multiwaits(m)
        return json.dumps(m).encode()

    nc.to_json_bytes = patched
    return nc


_NC_CACHE = None
last_run = None  # BassKernelResults of the most recent kernel() call


def _get_nc():
    global _NC_CACHE
    if _NC_CACHE is None:
        _NC_CACHE = _build_nc()
    return _NC_CACHE


def kernel(embeddings: np.ndarray, similarity_features: np.ndarray) -> np.ndarray:
    global last_run
    E = np.asarray(embeddings, dtype=np.float32)
    SF = np.asarray(similarity_features, dtype=np.float32)
    assert E.shape == (B, D) and SF.shape == (B, F)

    ET = np.ascontiguousarray(E.T).astype(ml_dtypes.bfloat16)        # [D, B]
    fn = SF / np.maximum(np.linalg.norm(SF, axis=1, keepdims=True), 1e-12)
    SFN = np.ascontiguousarray(fn.T).astype(ml_dtypes.bfloat16)      # [F, B]
    in_maps = []
    for c in range(NCORES):
        sh = c * RPC
        in_maps.append({
            "et": ET,
            "sfn": SFN,
            "etl": np.ascontiguousarray(ET[:, sh:sh + RPC]),
            "sfl": np.ascontiguousarray(SFN[:, sh:sh + RPC]),
        })

    nc = _get_nc()
    res = run_bass_kernel_spmd(nc, in_maps, core_ids=list(range(NCORES)))
    last_run = res

    # host combine: stat column idx = n4*NR + r; local row i = r*128 + p
    bce_num = np.zeros((NCORES, RPC), np.float64)
    pos_all = np.zeros((NCORES, RPC), np.float64)
    sgn_cols = np.array([_use_sgn(i) for i in range(NT)])
    for c, r in enumerate(res.results):
        def rows(a):
            # [128, NT] -> per-row sums over the NN chunks -> [RPC]
            return a.reshape(128, NN, NR).sum(axis=1).T.reshape(RPC)
        R = rows(r["out_r"].astype(np.float64))
        P = rows(r["out_p"].astype(np.float64))
        cc = r["out_c"].astype(np.float64)
        pos_chunks = np.where(sgn_cols[None, :], (CHUNK + cc) / 2.0, cc)
        bce_num[c] = INV_T * (R - P)
        pos_all[c] = rows(pos_chunks)

    bce_num = bce_num.reshape(-1)
    pos_off = pos_all.reshape(-1) - 1.0    # diagonal is always a positive
    neg_off = (B - 1) - pos_off
    row_loss = bce_num / np.float64(B - 1)
    valid = (pos_off >= 0.5) & (neg_off >= 0.5)
    num_valid = max(int(valid.sum()), 1)
    loss = np.float64(np.sum(np.where(valid, row_loss, 0.0))) / num_valid
    return np.float32(loss)
